# revision 52
# baseline (speedup 1.0000x reference)
"""Trainium2 Bass kernel for nn_MaskedPosmap2Normal.

Per batch image b and pixel (i,j), the reference computes
    d_k = neighbor_k - center  (k = right, up, left, down; zero-padded)
    normal = sum_k valid_k * (d_k x d_{k+1 mod 4})
    out = normal / max(||normal||, eps)
where valid_k is the AND of the 3 mask bits bracketing directions k, k+1.

Algebraic factorization (verified vs the reference, rel err 6.6e-5 in f32):
    y = m * x              (premasked posmap; m is the 0/1 mask)
    A = yR - yL - (mr - ml) * yC        (RL / "H" side)
    B = yU - yD - (mu - md) * yC        (UD / "G" side)
    normal = mc * (A x B)
i.e. ONE cross product, and every stencil term reads the premasked field y,
so the whole masked 4-direction diff collapses to 6 elementwise ops per
channel (A0, B0, arl, bud, A, B) plus the y premultiply.  mc is folded into
the reciprocal-norm scalar (rm = mc / ||n||), never touching the 3-channel
pipeline.

Precision strategy (numpy-validated on the real inputs; measured on real
TRN2: rel err 1.213e-3 vs the 2e-2 Frobenius harness gate):
  - y/A/B/ca/cb in fp16 -> every DVE tensor_tensor hits the 2x 16-bit mode
    (0.52 ns/elem vs 1.04 f32 measured in the cost model).
  - n = ca - cb on the TensorEngine (+/-identity f16 matmuls, f32 PSUM),
    so the catastrophic-cancellation subtraction is exact given its inputs.
  - sq = n^2 via ACT Square into BF16 (fp16 squares subnormal-flush below
    |n|~2e-4 and produced absmax ~3.7 garbage; bf16's range fixes that).
  - s = sum_c sq_c on TensorE (bf16 identity matmuls, f32 PSUM).
  - rsq = 1/sqrt(s + 1e-8) in ONE direct-emitted ACT Rsqrt (bass bans the
    LUT for precision, but its HW error is invisible at our gate and it
    replaces Sqrt + a 1x DVE reciprocal); rm = mc * rsq (f16 2x);
    o16 = n16 * rm (f16 2x); o = f32 widen on ACT; DMA out.
  - ACT uses only Copy/Square/Rsqrt = ONE act table
    ('reciprocal_sqrt_and_small'); ln/exp flip-flopped tables (1.38us per
    reload) with the greedy first-match table chooser.

Sharding: pure data parallel - one batch image per NeuronCore (8 cores).

Layout per core: partition p holds image rows [8p-1 .. 8p+8] (8 output rows
+ 1 halo row each side) so every stencil shift is a free-dim offset.
Columns are processed in chunks of CW with a 1-column halo (pitch P = CW+2).
Real HW charges ~780ns of issue overhead per DVE instruction (4.7x the cost
model - measured with the paired-reps bench), so ops are merged into
double-width tiles (A&B in one sub) and chunks are kept large and uniform.
The x loads ROTATE channels (tile slot s = true channel (s+1)%3, free at
DMA time): the AB tile then holds [A1 A2 A0 A1 | B1 B2 B0 B1] with slot 3
of each side a duplicate of slot 0 (one small ACT copy), which collapses
the six cross-product muls into TWO contiguous DVE ops - and the rotation
cancels inside the cross product (ca[s] = A_{s+1}B_{s+2} = natural-order
ca_s), so n/sq/o and the stores never unrotate.  DVE runs 10 instructions
per chunk (was 15).
The scalar tail (rm/o16/o) of chunk k is emitted after chunk k+1's cross
products (software pipelining), so DVE never idles on the PE->ACT->PE->ACT
norm chain; n is evacuated from PSUM to f16 SBUF by ACT so PSUM stays
single-buffered.  Output stores issue on the idle Pool SWDGE queue.

Halo x values at the image border are never zeroed: the resident f16 mask
tile IS zero there, so y = m*x kills them - the only requirement is that
x16 stays finite, hence the per-chunk Pool memsets of the halo-row slots
before the halo DMAs partially overwrite them.

The f16 mask field m16 and the precombined difference fields
mrl = mr - ml and mud = mu - md (values {-1,0,1}) stay SBUF-resident for
the whole image.

Rejected on real-HW evidence (sim-only wins):
  - GPSIMD elementwise offload: 2x slower + SBUF port contention with DVE.
  - DMA-CCE accumulation: wrong results.
  - fp16 ALU divide on DVE: neuronxcc ISA check rejects it (sim accepts).
  - SBUF->SBUF partition-shift DMA to build y's halo rows from the chunk's
    own freshly-written y tile: intermittently raced the DVE write on HW
    (NaN at rows = 0 mod 8, col 0) despite passing CoreSim's race detector.
fp16 rsq needs the 1e-8 floor: without it 1/sqrt overflows f16 on
fully-masked pixels.
"""

import os

import numpy as np

CH = 3
RPG = 8   # output rows per partition
NG = 10   # rows incl. halo
NCORES = 8

CW = int(os.environ.get("K_CW", "128"))
DEFER = os.environ.get("K_DEFER", "1") == "1"
OUTQ = os.environ.get("K_OUTQ", "pool")  # act | pool | sp
RSQRT = os.environ.get("K_RSQRT", "1") == "1"
# comma-separated op sites to run on GPSIMD instead of DVE: subset of
# {arl,bud,y,o16}.  Rejected for f32 in an earlier session (port contention);
# re-benchable on HW via bench.py.
GP_SITES = frozenset(x for x in os.environ.get("K_GP", "").split(",") if x)

_CACHE = {}


def _emit(ctx, tc, pm, mk, out, H, W, cw, reps=1):
    import concourse.bass as bass
    from concourse import mybir
    from concourse.masks import make_identity

    nc = tc.nc
    f32 = mybir.dt.float32
    f16 = mybir.dt.float16
    bf16 = mybir.dt.bfloat16
    u8 = mybir.dt.uint8
    AF = mybir.ActivationFunctionType
    ALU = mybir.AluOpType

    NP = H // RPG          # partitions used (128 at full size)
    P = cw + 2             # per-row pitch of a column-chunk x/y tile
    PM = W + 2             # per-row pitch of the resident mask tile
    NF = CH * RPG * cw     # fused free size of the A/B/ca/cb tiles
    SEG = RPG * cw         # per-channel block inside a fused tile
    nchunks = W // cw

    def bufs(name, dflt):
        return int(os.environ.get(f"K_B_{name}", str(dflt)))

    mres = ctx.enter_context(tc.tile_pool(name="mres", bufs=1))
    xin = ctx.enter_context(tc.tile_pool(name="xin", bufs=bufs("x", 2)))
    x16p = ctx.enter_context(tc.tile_pool(name="x16p", bufs=bufs("x16", 2)))
    ypool = ctx.enter_context(tc.tile_pool(name="ypool", bufs=bufs("y", 1)))
    wpool = ctx.enter_context(tc.tile_pool(name="wpool", bufs=bufs("w", 1)))
    sqpool = ctx.enter_context(tc.tile_pool(name="sqpool", bufs=bufs("sq", 1)))
    n16pool = ctx.enter_context(tc.tile_pool(name="n16p", bufs=bufs("n16", 2)))
    rpool = ctx.enter_context(tc.tile_pool(name="rpool", bufs=bufs("r", 2)))
    rpool1 = ctx.enter_context(tc.tile_pool(name="rpool1", bufs=bufs("r1", 1)))
    opool = ctx.enter_context(tc.tile_pool(name="opool", bufs=bufs("o", 1)))
    npsum = ctx.enter_context(tc.tile_pool(name="npsum", bufs=1, space="PSUM"))
    spsum = ctx.enter_context(tc.tile_pool(name="spsum", bufs=1, space="PSUM"))

    # ---- constants -------------------------------------------------------
    bias_eps = mres.tile([NP, 1], f32, name="bias_eps")
    nc.gpsimd.memset(bias_eps[:], 1e-8)

    idt32 = mres.tile([NP, NP], f32, name="idt32")
    make_identity(nc, idt32[:])
    idt16 = mres.tile([NP, NP], f16, name="idt16")
    nc.scalar.activation(idt16[:], idt32[:], AF.Copy)
    nidt16 = mres.tile([NP, NP], f16, name="nidt16")
    nc.vector.tensor_scalar_mul(nidt16[:], idt32[:], -1.0)
    idtbf = mres.tile([NP, NP], bf16, name="idtbf")
    nc.scalar.activation(idtbf[:], idt32[:], AF.Copy)
    # (Considered: folding mc into the norm via a 4th PE accumulation term
    # s += -BIG*mc undone by a +BIG rsqrt bias, to delete the rm DVE op.
    # Dead end: one f32 bias cannot hold BIG + the ~1e-9 floor (ulp), and
    # exactly-zero-norm valid pixels then hit Rsqrt's invalid range.)

    # ---- resident mask fields -------------------------------------------
    # m8/m16 slot (p, r, s) <-> mask[8p + r - 1, s - 1]; memset-0 first so
    # the never-DMA'd halo slots (image border) read as masked-out.
    m8 = mres.tile([NP, NG * PM], u8, name="m8")
    m8v = m8.rearrange("p (r q) -> p r q", r=NG)
    # zero only the halo slots the DMAs below don't write (col pads, and the
    # full row-0/row-9 slots whose interiors the halo DMAs then overwrite)
    nc.gpsimd.memset(m8v[:, :, 0:1], 0)
    nc.gpsimd.memset(m8v[:, :, PM - 1 : PM], 0)
    nc.gpsimd.memset(m8v[:, 0:1, :], 0)
    nc.gpsimd.memset(m8v[:, 9:10, :], 0)
    # central rows on SP; halo rows on the idle Pool queue so they don't
    # delay the central DMA the m16 conversion critical path needs
    src = bass.AP(mk, 0, [[RPG * W, NP], [W, RPG], [1, W]])
    nc.sync.dma_start(out=m8v[:, 1:9, 1 : 1 + W], in_=src)
    srct = bass.AP(mk, (RPG - 1) * W, [[RPG * W, NP - 1], [1, W]])
    nc.gpsimd.dma_start(out=m8v[1:NP, 0:1, 1 : 1 + W], in_=srct)
    srcb = bass.AP(mk, RPG * W, [[RPG * W, NP - 1], [1, W]])
    nc.gpsimd.dma_start(out=m8v[0 : NP - 1, 9:10, 1 : 1 + W], in_=srcb)

    # convert the central rows first: mrl (and chunk 0's y) only need rows
    # 1:9, so DVE can start while the 2 halo rows convert behind it
    m16 = mres.tile([NP, NG * PM], f16, name="m16")
    m16v = m16.rearrange("p (r q) -> p r q", r=NG)
    nc.scalar.activation(m16v[:, 1:9, :], m8v[:, 1:9, :], AF.Copy)

    # mrl[j] = m[i, j+1] - m[i, j-1]; mud[j] = m[i-1, j] - m[i+1, j]
    mrl = mres.tile([NP, RPG * W], f16, name="mrl")
    mrl3 = mrl.rearrange("p (r q) -> p r q", r=RPG)
    nc.vector.tensor_sub(mrl3, m16v[:, 1:9, 2 : 2 + W], m16v[:, 1:9, 0:W])

    nc.scalar.activation(m16v[:, 0:1, :], m8v[:, 0:1, :], AF.Copy)
    nc.scalar.activation(m16v[:, 9:10, :], m8v[:, 9:10, :], AF.Copy)
    mud = mres.tile([NP, RPG * W], f16, name="mud")
    mud3 = mud.rearrange("p (r q) -> p r q", r=RPG)
    nc.vector.tensor_sub(mud3, m16v[:, 0:8, 1 : 1 + W], m16v[:, 2:10, 1 : 1 + W])

    def bc3(view, q):  # [NP, 8, q] -> broadcast [NP, 3, 8, q]
        return view.unsqueeze(1).to_broadcast([NP, CH, RPG, q])

    def act_rsqrt(out_ap, in_ap):
        """out = 1/sqrt(in + eps) on ACT.  bass.activation() blanket-bans
        Rsqrt for precision; our Frobenius gate is 2e-2 and the HW run
        verifies the LUT accuracy, so emit the instruction directly."""
        eng = nc.scalar
        ins = [eng.lower_ap(in_ap), eng.lower_ap(bias_eps[:]),
               mybir.ImmediateValue(dtype=f32, value=1.0),
               mybir.ImmediateValue(dtype=f32, value=0.0)]
        eng.add_instruction(mybir.InstActivation(
            name=nc.get_next_instruction_name(),
            func=AF.Rsqrt, ins=ins, outs=[eng.lower_ap(out_ap)]))

    def emit_tail(pend):
        """Deferred DVE tail: runs one chunk behind so DVE never waits on
        the PE/ACT norm round trip."""
        k, j0, cwk, n16, t16 = pend
        NFk, SEGk = CH * RPG * cwk, RPG * cwk
        if RSQRT:
            rt = t16  # t16 already holds 1/sqrt(s + eps)
        else:
            rt = rpool1.tile([NP, SEGk], f16, name=f"rt_{k}", tag="rt")
            with nc.allow_low_precision(reason="bounded unit-scale scalar"):
                nc.vector.reciprocal(rt[:], t16[:])
        rm = rpool1.tile([NP, SEGk], f16, name=f"rm_{k}", tag="rm")
        rm3 = rm.rearrange("p (r q) -> p r q", r=RPG)
        nc.vector.tensor_tensor(rm3, m16v[:, 1:9, 1 + j0 : 1 + j0 + cwk],
                                rt.rearrange("p (r q) -> p r q", r=RPG),
                                ALU.mult)
        o16 = rpool1.tile([NP, NFk], f16, name=f"o16_{k}", tag="o16")
        rb = rm.unsqueeze(1).to_broadcast([NP, CH, SEGk])
        nc.vector.tensor_tensor(o16.rearrange("p (c q) -> p c q", c=CH),
                                n16.rearrange("p (c q) -> p c q", c=CH),
                                rb, ALU.mult)
        o = opool.tile([NP, NFk], f32, name=f"o_{k}", tag="o")
        nc.scalar.activation(o[:], o16[:], AF.Copy)
        o4 = o.rearrange("p (c r q) -> p c r q", c=CH, r=RPG)
        outq = {"act": nc.scalar, "pool": nc.gpsimd, "sp": nc.sync}[OUTQ]
        for c in range(CH):
            dst = bass.AP(out, c * H * W + j0,
                          [[RPG * W, NP], [W, RPG], [1, cwk]])
            outq.dma_start(out=dst, in_=o4[:, c])

    # uniform chunks: splitting the last chunk to shorten the final norm
    # round trip helped the cost model ~1us but costs ~11us on real HW
    # (each extra DVE instruction is ~780ns of measured issue overhead)
    plan = [(k0 * cw, cw) for k0 in range(nchunks)]
    pending = None
    for rep in range(reps):
      for k0, (j0, cwk) in enumerate(plan):
        k = rep * len(plan) + k0
        P = cwk + 2
        NF, SEG = CH * RPG * cwk, RPG * cwk
        lo = max(j0 - 1, 0)
        hi = min(j0 + cwk + 1, W)
        ncols = hi - lo
        soff = lo - (j0 - 1)

        # ---- x load: [NP, 3, NG, P] f32, 8 central rows + 1 halo row each
        # side from HBM.  (An SBUF-SBUF partition-shift DMA for the y halo
        # rows was 5% faster in the cost model but intermittently raced the
        # y write on real HW -> NaN at rows = 0 mod 8; HBM halos are safe.)
        xt = xin.tile([NP, CH * NG * P], f32, name=f"x_{k}", tag="x")
        xt4 = xt.rearrange("p (c r q) -> p c r q", c=CH, r=NG)
        # halo-row slots: zero first (keeps the never-DMA'd corner slots of
        # partitions 0 / NP-1 finite; m16 is 0 there so y ignores them),
        # then the halo DMAs overwrite the real parts.
        nc.gpsimd.memset(xt4[:, :, 0:1, :], 0.0)
        nc.gpsimd.memset(xt4[:, :, 9:10, :], 0.0)
        # one DMA per channel per row-band: a channel-merged halo DMA
        # ([partition, channel, col] with 4MB channel strides) produced
        # intermittent NaN at the halo rows on real HW - its completion
        # semaphore seems to fire before all strided writes land
        # channel ROTATION at load time: tile slot s holds true channel
        # (s+1)%3.  Downstream this makes both cross-product muls read
        # contiguous slot ranges (see the AB layout below), and the
        # rotation cancels in the cross product itself: ca[s] =
        # Arot[s]*Brot[s+1] = A_{s+1}*B_{s+2} = the natural-order ca_s,
        # so n/sq/o and the output stores never need unrotating.
        for s in range(CH):
            base = ((s + 1) % CH) * H * W
            tv = xt4[:, s]
            src = bass.AP(pm, base + lo, [[RPG * W, NP], [W, RPG], [1, ncols]])
            nc.sync.dma_start(out=tv[:, 1:9, soff : soff + ncols], in_=src)
            srct = bass.AP(pm, base + (RPG - 1) * W + lo,
                           [[RPG * W, NP - 1], [1, ncols]])
            nc.sync.dma_start(out=tv[1:NP, 0:1, soff : soff + ncols], in_=srct)
            srcb = bass.AP(pm, base + RPG * W + lo,
                           [[RPG * W, NP - 1], [1, ncols]])
            nc.sync.dma_start(out=tv[0 : NP - 1, 9:10, soff : soff + ncols],
                              in_=srcb)
        if soff > 0:
            nc.gpsimd.memset(xt4[:, :, :, 0:soff], 0.0)
        if soff + ncols < P:
            nc.gpsimd.memset(xt4[:, :, :, soff + ncols : P], 0.0)

        # ---- x -> f16 (ACT), y = m16 * x16 (DVE 2x) ---------------------
        x16 = x16p.tile([NP, CH * NG * P], f16, name=f"x16_{k}", tag="x16")
        nc.scalar.activation(x16[:], xt[:], AF.Copy)
        x16v = x16.rearrange("p (c r q) -> p c r q", c=CH, r=NG)

        y = ypool.tile([NP, CH * NG * P], f16, name=f"y_{k}", tag="y")
        y4 = y.rearrange("p (c r q) -> p c r q", c=CH, r=NG)
        mwin = m16v[:, :, j0 : j0 + P].unsqueeze(1).to_broadcast(
            [NP, CH, NG, P])
        nc.vector.tensor_tensor(y4, mwin, x16v, ALU.mult)

        yC = y4[:, :, 1:9, 1 : 1 + cwk]
        yR = y4[:, :, 1:9, 2 : 2 + cwk]
        yL = y4[:, :, 1:9, 0:cwk]
        yU = y4[:, :, 0:8, 1 : 1 + cwk]
        yD = y4[:, :, 2:10, 1 : 1 + cwk]

        def wt(nm, tag=None):
            return wpool.tile([NP, NF], f16, name=f"{nm}_{k}", tag=tag or nm)

        w4 = lambda t: t.rearrange("p (c r q) -> p c r q", c=CH, r=RPG)

        mrlv = bc3(mrl3[:, :, j0 : j0 + cwk], cwk)
        mudv = bc3(mud3[:, :, j0 : j0 + cwk], cwk)

        # w0/w1 are reused for the B side after A consumed them: the WAR is
        # resolved by DVE program order (same engine), costing no stall.
        def eng(site):
            return nc.gpsimd if site in GP_SITES else nc.vector

        # A0/B0 share one double-width tile (ab0) and arl/bud another, so
        # A = A0 - arl and B = B0 - bud collapse into ONE tensor_sub over
        # 2*NF: on real HW each DVE instruction costs ~780ns of issue
        # overhead (measured), so fewer/larger ops win.  With the rotated
        # channel slots (slot s = true channel s+1), the AB tile holds FOUR
        # slots per side [A1 A2 A0 A1 | B1 B2 B0 B1] - slot 3 of each side
        # is a duplicate of slot 0 (one small strided ACT copy) - and the
        # whole cross product collapses to TWO contiguous DVE muls:
        #   ca = Arot[0:3] * Brot[1:4] = AB[0:3S] * AB[5S:8S]
        #   cb = Arot[1:4] * Brot[0:3] = AB[1S:4S] * AB[4S:7S]
        ab0 = wpool.tile([NP, 2 * NF], f16, name=f"ab0_{k}", tag="w0")
        fld = wpool.tile([NP, 2 * NF], f16, name=f"fld_{k}", tag="w1")
        ab = wpool.tile([NP, 8 * SEG], f16, name=f"ab_{k}", tag="AB")
        a4 = lambda t, i: t[:, i * NF : (i + 1) * NF].rearrange(
            "p (c r q) -> p c r q", c=CH, r=RPG)
        nc.vector.tensor_sub(a4(ab0, 0), yR, yL)
        eng("arl").tensor_tensor(a4(fld, 0), mrlv, yC, ALU.mult)
        nc.vector.tensor_sub(a4(ab0, 1), yU, yD)
        eng("bud").tensor_tensor(a4(fld, 1), mudv, yC, ALU.mult)
        ab2 = lambda t: t.rearrange("p (s q) -> p s q", s=2)
        abm = ab.rearrange("p (s q) -> p s q", s=2)  # s: A/B side, 4*SEG
        nc.vector.tensor_sub(abm[:, :, 0 : 3 * SEG], ab2(ab0), ab2(fld))
        # duplicate slot 3 <- slot 0 of each side (one strided ACT copy).
        # NOT on GPSIMD: a Pool op in this dependency chain measured ~28us
        # per chunk on real HW (463us/image total) - 7x worse than modeled.
        nc.scalar.activation(abm[:, :, 3 * SEG : 4 * SEG],
                             abm[:, :, 0:SEG], AF.Copy)

        if DEFER and pending is not None:
            emit_tail(pending)  # covers the ACT dup-copy latency
            pending = None

        # ---- cross product: n = A x B (muls on DVE, sub on TensorE) -----
        # ca/cb reuse the (now dead) ab0 buffer: tag w0, DVE-order WAR
        cc = wpool.tile([NP, 2 * NF], f16, name=f"cc_{k}", tag="w0")
        ca = cc[:, 0:NF]
        cb = cc[:, NF : 2 * NF]
        nc.vector.tensor_tensor(ca, ab[:, 0 : 3 * SEG],
                                ab[:, 5 * SEG : 8 * SEG], ALU.mult)
        nc.vector.tensor_tensor(cb, ab[:, SEG : 4 * SEG],
                                ab[:, 4 * SEG : 7 * SEG], ALU.mult)

        n_ps = npsum.tile([NP, NF], f32, name=f"n_{k}", tag="n")
        for s0 in range(0, NF, 512):
            sw = min(512, NF - s0)
            nc.tensor.matmul(n_ps[:, s0 : s0 + sw], idt16[:],
                             ca[:, s0 : s0 + sw], start=True, stop=False)
            nc.tensor.matmul(n_ps[:, s0 : s0 + sw], nidt16[:],
                             cb[:, s0 : s0 + sw], start=False, stop=True)

        # ---- 1/||n||: sq(bf16) -> PE sum -> sqrt; n evacuated to f16 ----
        sq = sqpool.tile([NP, NF], bf16, name=f"sq_{k}", tag="sq")
        nc.scalar.activation(sq[:], n_ps[:], AF.Square)
        n16 = n16pool.tile([NP, NF], f16, name=f"n16_{k}", tag="n16")
        nc.scalar.activation(n16[:], n_ps[:], AF.Copy)
        s_ps = spsum.tile([NP, SEG], f32, name=f"s_{k}", tag="s")
        for s0 in range(0, SEG, 512):
            sw = min(512, SEG - s0)
            for c in range(CH):
                nc.tensor.matmul(s_ps[:, s0 : s0 + sw], idtbf[:],
                                 sq[:, c * SEG + s0 : c * SEG + s0 + sw],
                                 start=(c == 0), stop=(c == CH - 1))

        t16 = rpool.tile([NP, SEG], f16, name=f"t_{k}", tag="t")
        if RSQRT:
            act_rsqrt(t16[:], s_ps[:])
        else:
            nc.scalar.activation(t16[:], s_ps[:], AF.Sqrt, bias=bias_eps[:])

        if DEFER:
            if pending is not None:
                emit_tail(pending)
            pending = (k, j0, cwk, n16, t16)
        else:
            emit_tail((k, j0, cwk, n16, t16))

    if pending is not None:
        emit_tail(pending)


def build(H=1024, W=1024, cw=None, reps=1):
    cw = cw or CW
    key = (H, W, cw, reps)
    if key in _CACHE:
        return _CACHE[key]
    from contextlib import ExitStack

    import concourse.tile as tile
    from concourse import bacc, mybir

    nc = bacc.Bacc("TRN2", target_bir_lowering=False, debug=False,
                   num_devices=NCORES)
    pm = nc.dram_tensor("posmap", [CH, H, W], mybir.dt.float32,
                        kind="ExternalInput")
    mk = nc.dram_tensor("mask", [H, W], mybir.dt.uint8, kind="ExternalInput")
    out = nc.dram_tensor("out", [CH, H, W], mybir.dt.float32,
                         kind="ExternalOutput")
    with tile.TileContext(nc) as tc:
        with ExitStack() as ctx:
            _emit(ctx, tc, pm, mk, out, H, W, cw, reps)
    nc.compile()
    _CACHE[key] = nc
    return nc


def kernel(posmap: np.ndarray, mask: np.ndarray, _trace: bool = False):
    nc = build(posmap.shape[2], posmap.shape[3])
    from concourse.bass_utils import run_bass_kernel_spmd

    mask_u8 = np.ascontiguousarray(mask.astype(np.uint8))
    nb = posmap.shape[0]
    in_maps = [
        {"posmap": np.ascontiguousarray(posmap[b]), "mask": mask_u8}
        for b in range(nb)
    ]
    try:
        res = run_bass_kernel_spmd(nc, in_maps, core_ids=list(range(nb)),
                                   trace=_trace)
    except ModuleNotFoundError:
        res = run_bass_kernel_spmd(nc, in_maps, core_ids=list(range(nb)),
                                   trace=False)
    out = np.stack([res.results[b]["out"] for b in range(nb)], axis=0)
    if _trace:
        kernel.last_exec_time_ns = res.exec_time_ns
        kernel.last_trace = res.instructions_and_trace
    return out
